# revision 28
# baseline (speedup 1.0000x reference)
"""Binarized 3x3 conv (stride 1, pad 1) + bias on 8 Trainium2 NeuronCores.

Full problem: x[32,256,56,56] f32, weight[256,256,3,3] f32, bias[256] f32
-> y[32,256,56,56] f32 with y = conv2d(sign(x), sign(weight), pad=1) + bias
(sign(t) = +1 for t >= 0 else -1).

Sharding: data-parallel over batch. Each of the 8 cores gets 4 images and a
replicated copy of weight/bias, computes its shard fully on-device, and the
host concatenates the 8 output shards.

Per-core kernel (PE-bound: 504 fp8 DoubleRow matmuls at N=464 is 97.4us of
tensor-engine time at 2.4 GHz; everything else must overlap under it):
  - weight is marshalled on the host into the exact lhsT SBUF layout the
    matmuls need: sign(w)*0.5 as fp8 [128(ci_p), 2(ci_blk), 9(tap), 256(co)].
    A few strided DMAs load it; no on-device transposes.
  - x is binarized to +/-0.5 fp8 by DVE ((v>=0) - 0.5) into zero-padded
    SBUF images laid out [128(ci_p), 3376(padded elem), 2(ci_blk)]: 58x58
    padded rows + 1 guard elem front/back, with the two ci blocks
    INTERLEAVED at stride 1. This keeps every conv chunk's rhs access
    pattern inside a tight 929-byte window, so the Tile framework's
    bounding-interval dependency tracking gates each chunk only on the x
    rows it actually reads (instead of the whole image) and the conv
    stream starts as soon as the first 9 rows have landed.
  - conv: k-major over 8-row output chunks, co block inner, so each x row
    chunk is consumed immediately for both co blocks (halves the HBM rate
    the first image needs). Each chunk accumulates 9 DoubleRow fp8 matmuls
    (one per tap, K=256 packed as [128,2] adjacent bytes), M=128/N=464,
    into one PSUM bank (8-bank rotation). Outputs at the 2 pad columns of
    each row are garbage and are skipped on the way out.
  - PSUM -> SBUF via ScalarE: Identity(psum*4 + bias[co]) undoes the 0.25
    product scale, so results are exactly the +/-1 conv (ints, exact f32).
  - startup: weight/x DMA descriptors are issued from several engine
    queues in parallel; a few throwaway matmuls at t=0 start the PE
    p-state ramp early while the first x rows are in flight.
"""

import numpy as np

import concourse.bacc as bacc
import concourse.mybir as mybir
import concourse.tile as tile
from concourse.bass_utils import run_bass_kernel_spmd

F32 = mybir.dt.float32
BF16 = mybir.dt.bfloat16
FP8 = mybir.dt.float8e4
FP8_NP = mybir.dt.np(FP8)
AF = mybir.ActivationFunctionType
ALU = mybir.AluOpType
DR = mybir.MatmulPerfMode.DoubleRow

N_CORES = 8
H = W = 56
WP = 58            # padded row width
CIN = 256
COUT = 256
CI_BLKS = 2        # 256 ci = 2 x 128 partitions
CO_BLKS = 2
R = 8              # output rows per chunk
NCHUNK = H // R    # 7
NV = R * WP        # 464 matmul moving free size
IMG_FA = 3376      # per-ci_blk padded image elems (58*58+2 -> 3376)

# image-0 x row pieces: chunk k needs raw rows [8k-1, 8k+9)
P0_ROWS = (0, 9, 17, 25, 33, 41, 49, H)
# later images prefetch a whole conv-image ahead; 2 coarse pieces suffice
PN_ROWS = (0, 28, H)
N_JUNK = 14        # PE p-state warm-up matmuls bridging to the first x rows


def _build_conv(tc, y_ap, x_ap, wt_ap, b_ap, n_imgs):
    nc = tc.nc
    scale = 4.0  # undo (+/-0.5)*(+/-0.5) = +/-0.25 product scale

    with (
        tc.tile_pool(name="consts", bufs=1) as consts,
        tc.tile_pool(name="lhst", bufs=1) as lhst_pool,
        tc.tile_pool(name="xstage", bufs=3) as xstage_pool,
        tc.tile_pool(name="xpad", bufs=1) as xpad_pool,
        tc.tile_pool(name="outsb", bufs=6) as out_pool,
        tc.tile_pool(name="cpsum", bufs=8, space="PSUM") as cpsum_pool,
    ):
        junk = consts.tile([128, 512], BF16, name="junk")
        bias_sb = consts.tile([128, CO_BLKS], F32)
        lhst = lhst_pool.tile([128, CI_BLKS, 9, COUT], FP8)
        NXPAD = 3
        # [part, padded elem, ci_blk]: the two ci blocks interleaved
        xpads = [xpad_pool.tile([128, IMG_FA, CI_BLKS], FP8,
                                name=f"xpad{i}", tag=f"xpad{i}")
                 for i in range(NXPAD)]
        xstages = [xstage_pool.tile([128, CI_BLKS, H * W], F32,
                                    name=f"xstage{n}", tag="xstage")
                   for n in range(n_imgs)]

        def dma_x(eng, n, r0, r1, b):
            eng.dma_start(
                out=xstages[n][:, b, r0 * W:r1 * W],
                in_=x_ap[n, b * 128:(b + 1) * 128, r0:r1]
                    .rearrange("c h w -> c (h w)"),
            )

        def pad_memset(xp):
            # head guard + top pad row (+ first in-row pad col): elems 0..59
            nc.vector.memset(xp[:, 0:60, :], 0.0)
            # bottom pad row + tail guard: elems 1+57*58 .. 3375
            nc.vector.memset(xp[:, 1 + 57 * WP:IMG_FA, :], 0.0)
            # per-row right+left pad pairs at (1+h*58+57, 1+h*58+58)
            nc.vector.memset(
                xp[:, 58:58 + 57 * WP, :].rearrange(
                    "p (h w) b -> p h w b", w=WP)[:, :, 0:2, :],
                0.0,
            )

        def binz_x(n, r0, r1, b, eng=None):
            # data rows: padded row h+1, cols 1..56, interleaved blk b
            xpad = xpads[n % NXPAD]
            dst = xpad[:, 60:60 + H * WP, b:b + 1].rearrange(
                "p (h w) o -> p h (w o)", w=WP)[:, r0:r1, 0:W]
            src = xstages[n][:, b].rearrange("p (h w) -> p h w", w=W)[:, r0:r1]
            (eng or nc.vector).tensor_scalar(
                dst, src, 0.0, 0.5, ALU.is_ge, ALU.subtract)

        # --- DMA issue: the two x pieces gating conv chunk 0 lead both x
        # queues (queues transfer in order and share the 16 HW DMA
        # engines, so anything queued behind the x burst can starve);
        # weight pieces follow, spread over all three queues ordered by
        # tap need-time ---------------------------------------------------
        dma_x(nc.sync, 0, P0_ROWS[0], P0_ROWS[1], 0)
        dma_x(nc.gpsimd, 0, P0_ROWS[0], P0_ROWS[1], 1)
        nc.scalar.dma_start(out=bias_sb, in_=b_ap.rearrange("(b p) -> p b", p=128))
        nc.scalar.dma_start(out=lhst[:, :, 6:9], in_=wt_ap[:, :, 6:9])
        nc.scalar.dma_start(out=lhst[:, :, 2:4], in_=wt_ap[:, :, 2:4])
        nc.sync.dma_start(out=lhst[:, :, 0:2], in_=wt_ap[:, :, 0:2])
        nc.gpsimd.dma_start(out=lhst[:, :, 4:6], in_=wt_ap[:, :, 4:6])

        # vector: junk/pads, then binarize in arrival order; x piece
        # p+1 is issued one step ahead of its binarize
        nc.vector.memset(junk, 0.0)
        pad_memset(xpads[0])
        for p in range(len(P0_ROWS) - 1):
            binz_x(0, P0_ROWS[p], P0_ROWS[p + 1], 0)
            binz_x(0, P0_ROWS[p], P0_ROWS[p + 1], 1)
            if p + 1 < len(P0_ROWS) - 1:
                dma_x(nc.sync, 0, P0_ROWS[p + 1], P0_ROWS[p + 2], 0)
                dma_x(nc.gpsimd, 0, P0_ROWS[p + 1], P0_ROWS[p + 2], 1)
            if p == 1 and NXPAD > 1 and n_imgs > 1:
                pad_memset(xpads[1])
            if p == 3 and NXPAD > 2 and n_imgs > 2:
                pad_memset(xpads[2])

        # --- PE p-state ramp: throwaway matmuls on zeros keep the PE busy
        # during the initial DMA wait so the clock ramps before the real
        # stream starts
        for _ in range(N_JUNK):
            jps = cpsum_pool.tile([128, 512], F32, name="ps", tag="ps")
            nc.tensor.matmul(jps, junk[:, :128], junk, start=True, stop=True)

        def conv_mms(n, c, k, r0, nr, ps):
            # 9-tap matmul accumulation into ps for output rows
            # [R*k+r0, ..+nr) of co block c; ps covers nr padded rows
            xpad = xpads[n % NXPAD]
            nv = nr * WP
            for t in range(9):
                kh, kw = divmod(t, 3)
                base = (R * k + r0 + kh) * WP + kw  # incl. -1 guard shift
                nc.tensor.matmul(
                    ps,
                    lhst[:, 0:2, t, c * 128:(c + 1) * 128],
                    xpad[:, base:base + nv, :].rearrange("p n b -> p b n"),
                    start=(t == 0),
                    stop=(t == 8),
                    perf_mode=DR,
                )

        def act_chunk(c, ps, osb_view):
            # ACT and the store issues both live on ScalarE: the y stream
            # gets its own DMA queue (x prefetch owns sync/gpsimd queues),
            # and ACT -> store needs no cross-engine semaphore
            nc.scalar.activation(
                out=osb_view.rearrange("p (r w) -> p r w", w=W),
                in_=ps.rearrange("p (r w) -> p r w", w=WP)[:, :, 1:57],
                func=AF.Identity,
                bias=bias_sb[:, c:c + 1],
                scale=scale,
            )

        def conv_chunk_pair(n, k):
            # both co blocks of row chunk k -> one [128, 2, 448] store
            # (halves the store count: fewer semaphore-ring collisions
            # with the x prefetch queues, fatter DMA lines)
            osb = out_pool.tile([128, CO_BLKS * R * W], F32, name="osb")
            for c in range(CO_BLKS):
                ps = cpsum_pool.tile([128, R * WP], F32, name="ps", tag="ps")
                conv_mms(n, c, k, 0, R, ps)
                act_chunk(c, ps, osb[:, c * R * W:(c + 1) * R * W])
            dst = y_ap[n].rearrange("(c p) h w -> p c (h w)", c=CO_BLKS) \
                [:, :, R * W * k:R * W * (k + 1)]
            nc.scalar.dma_start(
                out=dst, in_=osb.rearrange("p (c n) -> p c n", c=CO_BLKS))

        def conv_chunk_single(n, c, k, r0, nr):
            ps = cpsum_pool.tile([128, nr * WP], F32, name="ps", tag="ps")
            conv_mms(n, c, k, r0, nr, ps)
            osb = out_pool.tile([128, nr * W], F32, name="osb")
            act_chunk(c, ps, osb)
            y_rows = y_ap[n, c * 128:(c + 1) * 128] \
                .rearrange("co h w -> co (h w)") \
                [:, W * (R * k + r0):W * (R * k + r0 + nr)]
            nc.scalar.dma_start(out=y_rows, in_=osb)

        def load_image(n):
            for p in range(len(PN_ROWS) - 1):
                dma_x(nc.sync, n, PN_ROWS[p], PN_ROWS[p + 1], 0)
                dma_x(nc.gpsimd, n, PN_ROWS[p], PN_ROWS[p + 1], 1)
                binz_x(n, PN_ROWS[p], PN_ROWS[p + 1], 0)
                binz_x(n, PN_ROWS[p], PN_ROWS[p + 1], 1)

        for n in range(n_imgs):
            # prefetch image n+1 before image n's conv chunks so its input
            # DMAs take queue priority over image n's output-DMA burst
            if n + 1 < n_imgs:
                load_image(n + 1)
            for k in range(NCHUNK):
                if n == n_imgs - 1 and k == NCHUNK - 1:
                    # split the final chunk so the last store is small and
                    # its ACT/DMA overlap the second half's matmuls
                    conv_chunk_single(n, 0, k, 0, R)
                    conv_chunk_single(n, 1, k, 0, R // 2)
                    conv_chunk_single(n, 1, k, R // 2, R // 2)
                else:
                    conv_chunk_pair(n, k)


_NC_CACHE = {}


def _get_nc(n_imgs):
    if n_imgs not in _NC_CACHE:
        nc = bacc.Bacc("TRN2", target_bir_lowering=False, debug=False)
        x_ap = nc.dram_tensor("x", [n_imgs, CIN, H, W], F32,
                              kind="ExternalInput").ap()
        wt_ap = nc.dram_tensor("wt", [128, CI_BLKS, 9, COUT], FP8,
                               kind="ExternalInput").ap()
        b_ap = nc.dram_tensor("bias", [COUT], F32, kind="ExternalInput").ap()
        y_ap = nc.dram_tensor("y", [n_imgs, COUT, H, W], F32,
                              kind="ExternalOutput").ap()
        with tile.TileContext(nc) as tc:
            _build_conv(tc, y_ap, x_ap, wt_ap, b_ap, n_imgs)
        nc.compile()
        _NC_CACHE[n_imgs] = nc
    return _NC_CACHE[n_imgs]


def _prep_wt(weight):
    # sign(w)*0.5 as fp8 in the lhsT layout [ci_p, ci_blk, tap, co_global]
    w = weight.reshape(COUT, CI_BLKS, 128, 9)              # [co, b, ci_p, t]
    s = np.where(w >= 0, np.float32(0.5), np.float32(-0.5))
    return np.ascontiguousarray(
        s.transpose(2, 1, 3, 0)).astype(FP8_NP)            # [ci_p, b, t, co]


def _make_run_args(x, weight, bias):
    n_imgs = x.shape[0] // N_CORES
    x = np.ascontiguousarray(x, dtype=np.float32)
    wt = _prep_wt(np.ascontiguousarray(weight, dtype=np.float32))
    bias = np.ascontiguousarray(bias, dtype=np.float32)
    nc = _get_nc(n_imgs)
    shards = [x[i * n_imgs:(i + 1) * n_imgs] for i in range(N_CORES)]
    in_maps = [{"x": s, "wt": wt, "bias": bias} for s in shards]
    return nc, in_maps


def kernel(x: np.ndarray, weight: np.ndarray, bias: np.ndarray) -> np.ndarray:
    assert x.shape[1:] == (CIN, H, W), x.shape
    assert x.shape[0] % N_CORES == 0, x.shape
    nc, in_maps = _make_run_args(x, weight, bias)
    res = run_bass_kernel_spmd(nc, in_maps, core_ids=list(range(N_CORES)))
    return np.concatenate([r["y"] for r in res.results], axis=0)


# revision 30
# speedup vs baseline: 1.0038x; 1.0038x over previous
"""Binarized 3x3 conv (stride 1, pad 1) + bias on 8 Trainium2 NeuronCores.

Full problem: x[32,256,56,56] f32, weight[256,256,3,3] f32, bias[256] f32
-> y[32,256,56,56] f32 with y = conv2d(sign(x), sign(weight), pad=1) + bias
(sign(t) = +1 for t >= 0 else -1).

Sharding: data-parallel over batch. Each of the 8 cores gets 4 images and a
replicated copy of weight/bias, computes its shard fully on-device, and the
host concatenates the 8 output shards.

Per-core kernel (PE-bound: 504 fp8 DoubleRow matmuls at N=464 is 97.4us of
tensor-engine time at 2.4 GHz; everything else must overlap under it):
  - weight is marshalled on the host into the exact lhsT SBUF layout the
    matmuls need: sign(w)*0.5 as fp8 [128(ci_p), 2(ci_blk), 9(tap), 256(co)].
    A few strided DMAs load it; no on-device transposes.
  - x is binarized to +/-0.5 fp8 by DVE ((v>=0) - 0.5) into zero-padded
    SBUF images laid out [128(ci_p), 3376(padded elem), 2(ci_blk)]: 58x58
    padded rows + 1 guard elem front/back, with the two ci blocks
    INTERLEAVED at stride 1. This keeps every conv chunk's rhs access
    pattern inside a tight 929-byte window, so the Tile framework's
    bounding-interval dependency tracking gates each chunk only on the x
    rows it actually reads (instead of the whole image) and the conv
    stream starts as soon as the first 9 rows have landed.
  - conv: k-major over 8-row output chunks, co block inner, so each x row
    chunk is consumed immediately for both co blocks (halves the HBM rate
    the first image needs). Each chunk accumulates 9 DoubleRow fp8 matmuls
    (one per tap, K=256 packed as [128,2] adjacent bytes), M=128/N=464,
    into one PSUM bank (8-bank rotation). Outputs at the 2 pad columns of
    each row are garbage and are skipped on the way out.
  - PSUM -> SBUF via ScalarE: Identity(psum*4 + bias[co]) undoes the 0.25
    product scale, so results are exactly the +/-1 conv (ints, exact f32).
  - startup: weight/x DMA descriptors are issued from several engine
    queues in parallel; a few throwaway matmuls at t=0 start the PE
    p-state ramp early while the first x rows are in flight.
"""

import numpy as np

import concourse.bacc as bacc
import concourse.mybir as mybir
import concourse.tile as tile
from concourse.bass_utils import run_bass_kernel_spmd

F32 = mybir.dt.float32
BF16 = mybir.dt.bfloat16
FP8 = mybir.dt.float8e4
FP8_NP = mybir.dt.np(FP8)
AF = mybir.ActivationFunctionType
ALU = mybir.AluOpType
DR = mybir.MatmulPerfMode.DoubleRow

N_CORES = 8
H = W = 56
WP = 58            # padded row width
CIN = 256
COUT = 256
CI_BLKS = 2        # 256 ci = 2 x 128 partitions
CO_BLKS = 2
R = 8              # output rows per chunk
NCHUNK = H // R    # 7
NV = R * WP        # 464 matmul moving free size
IMG_FA = 3376      # per-ci_blk padded image elems (58*58+2 -> 3376)

# image-0 x row pieces: chunk k needs raw rows [8k-1, 8k+9)
P0_ROWS = (0, 9, 17, 25, 33, 41, 49, H)
# later images prefetch a whole conv-image ahead; 2 coarse pieces suffice
PN_ROWS = (0, 28, H)
N_JUNK = 14        # PE p-state warm-up matmuls bridging to the first x rows


def _build_conv(tc, y_ap, x_ap, wt_ap, b_ap, n_imgs):
    nc = tc.nc
    scale = 4.0  # undo (+/-0.5)*(+/-0.5) = +/-0.25 product scale

    with (
        tc.tile_pool(name="consts", bufs=1) as consts,
        tc.tile_pool(name="lhst", bufs=1) as lhst_pool,
        tc.tile_pool(name="xstage", bufs=3) as xstage_pool,
        tc.tile_pool(name="xpad", bufs=1) as xpad_pool,
        tc.tile_pool(name="outsb", bufs=6) as out_pool,
        tc.tile_pool(name="cpsum", bufs=8, space="PSUM") as cpsum_pool,
    ):
        junk = consts.tile([128, 512], BF16, name="junk")
        bias_sb = consts.tile([128, CO_BLKS], F32)
        lhst = lhst_pool.tile([128, CI_BLKS, 9, COUT], FP8)
        NXPAD = 3
        # [part, padded elem, ci_blk]: the two ci blocks interleaved
        xpads = [xpad_pool.tile([128, IMG_FA, CI_BLKS], FP8,
                                name=f"xpad{i}", tag=f"xpad{i}")
                 for i in range(NXPAD)]
        xstages = [xstage_pool.tile([128, CI_BLKS, H * W], F32,
                                    name=f"xstage{n}", tag="xstage")
                   for n in range(n_imgs)]

        def dma_x(eng, n, r0, r1, b):
            eng.dma_start(
                out=xstages[n][:, b, r0 * W:r1 * W],
                in_=x_ap[n, b * 128:(b + 1) * 128, r0:r1]
                    .rearrange("c h w -> c (h w)"),
            )

        def pad_memset(xp):
            # head guard + top pad row (+ first in-row pad col): elems 0..59
            nc.vector.memset(xp[:, 0:60, :], 0.0)
            # bottom pad row + tail guard: elems 1+57*58 .. 3375
            nc.vector.memset(xp[:, 1 + 57 * WP:IMG_FA, :], 0.0)
            # per-row right+left pad pairs at (1+h*58+57, 1+h*58+58)
            nc.vector.memset(
                xp[:, 58:58 + 57 * WP, :].rearrange(
                    "p (h w) b -> p h w b", w=WP)[:, :, 0:2, :],
                0.0,
            )

        def binz_x(n, r0, r1, b, eng=None):
            # data rows: padded row h+1, cols 1..56, interleaved blk b
            xpad = xpads[n % NXPAD]
            dst = xpad[:, 60:60 + H * WP, b:b + 1].rearrange(
                "p (h w) o -> p h (w o)", w=WP)[:, r0:r1, 0:W]
            src = xstages[n][:, b].rearrange("p (h w) -> p h w", w=W)[:, r0:r1]
            (eng or nc.vector).tensor_scalar(
                dst, src, 0.0, 0.5, ALU.is_ge, ALU.subtract)

        # --- DMA issue: the two x pieces gating conv chunk 0 lead both x
        # queues (queues transfer in order and share the 16 HW DMA
        # engines, so anything queued behind the x burst can starve);
        # weight pieces follow, spread over all three queues ordered by
        # tap need-time ---------------------------------------------------
        dma_x(nc.sync, 0, P0_ROWS[0], P0_ROWS[1], 0)
        dma_x(nc.gpsimd, 0, P0_ROWS[0], P0_ROWS[1], 1)
        nc.scalar.dma_start(out=bias_sb, in_=b_ap.rearrange("(b p) -> p b", p=128))
        nc.scalar.dma_start(out=lhst[:, :, 6:9], in_=wt_ap[:, :, 6:9])
        nc.sync.dma_start(out=lhst[:, :, 0:2], in_=wt_ap[:, :, 0:2])
        nc.sync.dma_start(out=lhst[:, :, 2:4], in_=wt_ap[:, :, 2:4])
        nc.gpsimd.dma_start(out=lhst[:, :, 4:6], in_=wt_ap[:, :, 4:6])

        # vector: junk/pads, then binarize in arrival order; x piece
        # p+1 is issued one step ahead of its binarize
        nc.vector.memset(junk, 0.0)
        pad_memset(xpads[0])
        for p in range(len(P0_ROWS) - 1):
            binz_x(0, P0_ROWS[p], P0_ROWS[p + 1], 0)
            binz_x(0, P0_ROWS[p], P0_ROWS[p + 1], 1)
            if p + 1 < len(P0_ROWS) - 1:
                dma_x(nc.sync, 0, P0_ROWS[p + 1], P0_ROWS[p + 2], 0)
                dma_x(nc.gpsimd, 0, P0_ROWS[p + 1], P0_ROWS[p + 2], 1)
            if p == 1 and NXPAD > 1 and n_imgs > 1:
                pad_memset(xpads[1])
            if p == 3 and NXPAD > 2 and n_imgs > 2:
                pad_memset(xpads[2])

        # --- PE p-state ramp: throwaway matmuls on zeros keep the PE busy
        # during the initial DMA wait so the clock ramps before the real
        # stream starts
        for _ in range(N_JUNK):
            jps = cpsum_pool.tile([128, 512], F32, name="ps", tag="ps")
            nc.tensor.matmul(jps, junk[:, :128], junk, start=True, stop=True)

        def conv_mms(n, c, k, r0, nr, ps):
            # 9-tap matmul accumulation into ps for output rows
            # [R*k+r0, ..+nr) of co block c; ps covers nr padded rows
            xpad = xpads[n % NXPAD]
            nv = nr * WP
            for t in range(9):
                kh, kw = divmod(t, 3)
                base = (R * k + r0 + kh) * WP + kw  # incl. -1 guard shift
                nc.tensor.matmul(
                    ps,
                    lhst[:, 0:2, t, c * 128:(c + 1) * 128],
                    xpad[:, base:base + nv, :].rearrange("p n b -> p b n"),
                    start=(t == 0),
                    stop=(t == 8),
                    perf_mode=DR,
                )

        def act_chunk(c, ps, osb_view):
            # ACT and the store issues both live on ScalarE: the y stream
            # gets its own DMA queue (x prefetch owns sync/gpsimd queues),
            # and ACT -> store needs no cross-engine semaphore
            nc.scalar.activation(
                out=osb_view.rearrange("p (r w) -> p r w", w=W),
                in_=ps.rearrange("p (r w) -> p r w", w=WP)[:, :, 1:57],
                func=AF.Identity,
                bias=bias_sb[:, c:c + 1],
                scale=scale,
            )

        def conv_chunk_pair(n, k):
            # both co blocks of row chunk k -> one [128, 2, 448] store
            # (halves the store count: fewer semaphore-ring collisions
            # with the x prefetch queues, fatter DMA lines)
            osb = out_pool.tile([128, CO_BLKS * R * W], F32, name="osb")
            for c in range(CO_BLKS):
                ps = cpsum_pool.tile([128, R * WP], F32, name="ps", tag="ps")
                conv_mms(n, c, k, 0, R, ps)
                act_chunk(c, ps, osb[:, c * R * W:(c + 1) * R * W])
            dst = y_ap[n].rearrange("(c p) h w -> p c (h w)", c=CO_BLKS) \
                [:, :, R * W * k:R * W * (k + 1)]
            nc.scalar.dma_start(
                out=dst, in_=osb.rearrange("p (c n) -> p c n", c=CO_BLKS))

        def conv_chunk_single(n, c, k, r0, nr):
            ps = cpsum_pool.tile([128, nr * WP], F32, name="ps", tag="ps")
            conv_mms(n, c, k, r0, nr, ps)
            osb = out_pool.tile([128, nr * W], F32, name="osb")
            act_chunk(c, ps, osb)
            y_rows = y_ap[n, c * 128:(c + 1) * 128] \
                .rearrange("co h w -> co (h w)") \
                [:, W * (R * k + r0):W * (R * k + r0 + nr)]
            nc.scalar.dma_start(out=y_rows, in_=osb)

        def load_image(n):
            for p in range(len(PN_ROWS) - 1):
                dma_x(nc.sync, n, PN_ROWS[p], PN_ROWS[p + 1], 0)
                dma_x(nc.gpsimd, n, PN_ROWS[p], PN_ROWS[p + 1], 1)
                binz_x(n, PN_ROWS[p], PN_ROWS[p + 1], 0)
                binz_x(n, PN_ROWS[p], PN_ROWS[p + 1], 1)

        for n in range(n_imgs):
            # prefetch image n+1 before image n's conv chunks so its input
            # DMAs take queue priority over image n's output-DMA burst
            if n + 1 < n_imgs:
                load_image(n + 1)
            for k in range(NCHUNK):
                if n == n_imgs - 1 and k == NCHUNK - 1:
                    # split the final chunk so the last store is tiny and
                    # its ACT/DMA overlap the earlier pieces' matmuls
                    conv_chunk_single(n, 0, k, 0, R)
                    conv_chunk_single(n, 1, k, 0, R // 2)
                    conv_chunk_single(n, 1, k, R // 2, R // 4)
                    conv_chunk_single(n, 1, k, 3 * R // 4, R // 4)
                else:
                    conv_chunk_pair(n, k)


_NC_CACHE = {}


def _get_nc(n_imgs):
    if n_imgs not in _NC_CACHE:
        nc = bacc.Bacc("TRN2", target_bir_lowering=False, debug=False)
        x_ap = nc.dram_tensor("x", [n_imgs, CIN, H, W], F32,
                              kind="ExternalInput").ap()
        wt_ap = nc.dram_tensor("wt", [128, CI_BLKS, 9, COUT], FP8,
                               kind="ExternalInput").ap()
        b_ap = nc.dram_tensor("bias", [COUT], F32, kind="ExternalInput").ap()
        y_ap = nc.dram_tensor("y", [n_imgs, COUT, H, W], F32,
                              kind="ExternalOutput").ap()
        with tile.TileContext(nc) as tc:
            _build_conv(tc, y_ap, x_ap, wt_ap, b_ap, n_imgs)
        nc.compile()
        _NC_CACHE[n_imgs] = nc
    return _NC_CACHE[n_imgs]


def _prep_wt(weight):
    # sign(w)*0.5 as fp8 in the lhsT layout [ci_p, ci_blk, tap, co_global]
    w = weight.reshape(COUT, CI_BLKS, 128, 9)              # [co, b, ci_p, t]
    s = np.where(w >= 0, np.float32(0.5), np.float32(-0.5))
    return np.ascontiguousarray(
        s.transpose(2, 1, 3, 0)).astype(FP8_NP)            # [ci_p, b, t, co]


def _make_run_args(x, weight, bias):
    n_imgs = x.shape[0] // N_CORES
    x = np.ascontiguousarray(x, dtype=np.float32)
    wt = _prep_wt(np.ascontiguousarray(weight, dtype=np.float32))
    bias = np.ascontiguousarray(bias, dtype=np.float32)
    nc = _get_nc(n_imgs)
    shards = [x[i * n_imgs:(i + 1) * n_imgs] for i in range(N_CORES)]
    in_maps = [{"x": s, "wt": wt, "bias": bias} for s in shards]
    return nc, in_maps


def kernel(x: np.ndarray, weight: np.ndarray, bias: np.ndarray) -> np.ndarray:
    assert x.shape[1:] == (CIN, H, W), x.shape
    assert x.shape[0] % N_CORES == 0, x.shape
    nc, in_maps = _make_run_args(x, weight, bias)
    res = run_bass_kernel_spmd(nc, in_maps, core_ids=list(range(N_CORES)))
    return np.concatenate([r["y"] for r in res.results], axis=0)
